# revision 39
# baseline (speedup 1.0000x reference)
"""Trainium2 Bass kernel for nn_EquiConv (e3nn-style tensor product with
per-edge generated weights), data-parallel over edges on 8 NeuronCores.

Per 1024-edge tile (8 blocks of 128 edges on partitions):
  PE : fwT = transpose(fw) (bf16); hT = W1n^T @ fwT (bf16); per block:
       w = hT-slice^T @ W2n' (bf16, 1024-wide moving) -> PSUM f32
  ACT: h = silu(hT); evacuate w PSUM -> SBUF bf16 (2x 1152 per block)
  DVE: tile-batched broadcast-muls + bf16 fold-trees (2x mode), per-edge
       factors via step-0 broadcast APs; all path constants folded into W2n'.
  GPS: path101 product + first fold level (offload from DVE).

W2n' host prep: scaled by SILU_NORM/sqrt(64), per-path constants
(pw00, pw110/sqrt3, pw011/sqrt3, pw101/sqrt3) folded into the respective
column blocks, then permuted to w-major order per path, with path011 and
path101 FIRST so they only depend on the first eviction chunk (cols
0:1152) and their DVE chains can start before the tile's w00 evictions
finish:
  path011   : col w*32 + u          (cols    0: 512)
  path101   : 512 + w*16 + u        (cols  512: 768)
  path00/110: 768 + w*48 + u        (cols  768:2304; u<32 -> w00 else w110)
"""
import math

import numpy as np

E_TOTAL = 65536
N_CORES = 8
E_CORE = E_TOTAL // N_CORES        # 8192
TILE_E = 1024
BLK = 128
NB = TILE_E // BLK                 # 8 blocks per tile
N_TILES = E_CORE // TILE_E         # 8
MUL0, MUL1 = 32, 16
FC_IN, FC_HID = 64, 64
WNUMEL = 2304
SILU_NORM = 1.6790
ISQRT3 = 1.0 / math.sqrt(3.0)
PW00 = math.sqrt(1.0 / (MUL0 * 2))
PW110I3 = math.sqrt(1.0 / (MUL1 * 2)) * ISQRT3
PW011I3 = math.sqrt(3.0 / (MUL0 * 2)) * ISQRT3
PW101I3 = math.sqrt(3.0 / (MUL1 * 2)) * ISQRT3

GPS_T101 = False                   # gpsimd TT measured ~6x slower than DVE; keep off

_NC_CACHE = {}


def _w2_prep(W2):
    """Scale + fold path constants + permute to w-major. Returns [64, 2304]."""
    W2n = W2.astype(np.float64) * (SILU_NORM / math.sqrt(FC_HID))
    W2n[:, 0:1024] *= PW00
    W2n[:, 1024:1536] *= PW110I3
    W2n[:, 1536:2048] *= PW011I3
    W2n[:, 2048:2304] *= PW101I3
    old = np.empty(WNUMEL, np.int64)
    for w in range(16):
        for u in range(32):
            old[w * 32 + u] = 1536 + u * 16 + w
    for w in range(16):
        for u in range(16):
            old[512 + w * 16 + u] = 2048 + u * 16 + w
    for w in range(32):
        for u in range(48):
            old[768 + w * 48 + u] = (u * 32 + w) if u < 32 \
                else (1024 + (u - 32) * 32 + w)
    return np.ascontiguousarray(W2n[:, old]).astype(np.float32)


def _build():
    import concourse.tile as tile
    from concourse import bacc, mybir
    from concourse.masks import make_identity

    f32 = mybir.dt.float32
    bf16 = mybir.dt.bfloat16
    MULT = mybir.AluOpType.mult
    ADD = mybir.AluOpType.add
    AXX = mybir.AxisListType.X

    nc = bacc.Bacc("TRN2", target_bir_lowering=False, debug=False)
    fea_in1 = nc.declare_dram_parameter("fea_in1", [E_CORE, 80], f32, isOutput=False)
    fea_in2 = nc.declare_dram_parameter("fea_in2", [E_CORE, 4], f32, isOutput=False)
    fea_w = nc.declare_dram_parameter("fea_weight", [E_CORE, 64], f32, isOutput=False)
    W1n = nc.declare_dram_parameter("W1n", [64, 64], f32, isOutput=False)
    W2n = nc.declare_dram_parameter("W2n", [64, WNUMEL], f32, isOutput=False)
    out_d = nc.declare_dram_parameter("out", [E_CORE, 80], f32, isOutput=True)

    with tile.TileContext(nc) as tc, nc.allow_low_precision("bf16 contraction"):
        with (
            tc.tile_pool(name="consts", bufs=1) as consts,
            tc.tile_pool(name="ins", bufs=3) as insp,
            tc.tile_pool(name="mid", bufs=2) as mid,
            tc.tile_pool(name="wsb", bufs=2) as wsbp,
            tc.tile_pool(name="work", bufs=3) as work,
            tc.tile_pool(name="tmp", bufs=1) as tmpp,
            tc.tile_pool(name="outs", bufs=2) as outsp,
            tc.tile_pool(name="ps_w", bufs=2, space="PSUM") as ps_w,
            tc.tile_pool(name="ps_s", bufs=1, space="PSUM") as ps_s,
        ):
            # ident first: it gates the PE warm-up, so it must not queue
            # behind the large W2n cast-DMA on the gpsimd queue
            ident = consts.tile([128, 128], f32)
            make_identity(nc, ident)
            w1_t = consts.tile([64, 64], bf16)
            nc.gpsimd.dma_start(w1_t[:], W1n[:])
            # W2n load split so mm2's first chunk (cols 0:1152) is usable
            # before the whole matrix lands
            w2_t = consts.tile([64, WNUMEL], bf16)
            nc.gpsimd.dma_start(w2_t[:, 0:1152], W2n[:, 0:1152])
            nc.gpsimd.dma_start(w2_t[:, 1152:WNUMEL], W2n[:, 1152:WNUMEL])

            # PE warm-up: start PE activity early to un-throttle the HAM
            # clock gate (1.2 -> 2.4 GHz); tile 0's real transposes continue
            # the activity stream, so a short burst suffices.
            warm = ps_s.tile([64, 512], f32, tag="fwT")
            for i in range(10):
                nc.tensor.transpose(
                    warm[:, (i % 4) * 128:(i % 4 + 1) * 128],
                    ident[:, 0:64], ident[:])

            for t in range(N_TILES):
                e0 = t * TILE_E
                # ---- batched input loads (bf16 via DMA convert) ----
                fwB = insp.tile([BLK, NB * 64], f32, tag="fwB")
                nc.sync.dma_start(
                    fwB[:].rearrange("p (b f) -> p b f", b=NB),
                    fea_w[e0:e0 + TILE_E].rearrange("(b p) f -> p b f", p=BLK))
                x1B = insp.tile([BLK, NB * 80], bf16, tag="x1B")
                nc.gpsimd.dma_start(
                    x1B[:].rearrange("p (b f) -> p b f", b=NB),
                    fea_in1[e0:e0 + TILE_E].rearrange("(b p) f -> p b f", p=BLK))
                x2B = insp.tile([BLK, NB * 4], f32, tag="x2B")
                nc.sync.dma_start(
                    x2B[:].rearrange("p (b f) -> p b f", b=NB),
                    fea_in2[e0:e0 + TILE_E].rearrange("(b p) f -> p b f", p=BLK))

                # views over the batched per-edge features
                x1v = x1B[:].rearrange("p (b f) -> p b f", b=NB)    # [128,8,80]
                x2v = x2B[:].rearrange("p (b f) -> p b f", b=NB)    # [128,8,4]

                # ---- fw transpose + mm1 + silu (per 512-half) ----
                fwT_sb = mid.tile([64, TILE_E], bf16, tag="fwT_sb")
                h_sb = mid.tile([64, TILE_E], bf16, tag="h_sb")
                for hf in range(TILE_E // 512):
                    fwT_ps = ps_s.tile([64, 512], f32, tag="fwT")
                    for b in range(4):
                        nc.tensor.transpose(
                            fwT_ps[:, b * BLK:(b + 1) * BLK],
                            fwB[:, (hf * 4 + b) * 64:(hf * 4 + b + 1) * 64],
                            ident[:])
                    nc.scalar.copy(
                        fwT_sb[:, hf * 512:(hf + 1) * 512], fwT_ps[:])
                    h_ps = ps_s.tile([64, 512], f32, tag="h")
                    nc.tensor.matmul(
                        h_ps[:], w1_t[:], fwT_sb[:, hf * 512:(hf + 1) * 512],
                        start=True, stop=True)
                    nc.scalar.activation(
                        h_sb[:, hf * 512:(hf + 1) * 512], h_ps[:],
                        mybir.ActivationFunctionType.Silu)

                # ---- mm2 (bf16, 1024-wide moving) + eviction (2x 1152) ----
                w_sb = wsbp.tile([BLK, NB * WNUMEL], bf16, tag="w_sb")
                for b in range(NB):
                    lhs = h_sb[:, b * BLK:(b + 1) * BLK]
                    for half in range(2):
                        wp = ps_w.tile([BLK, 1152], f32, tag="wp")
                        c0 = half * 1152
                        for s0, s1 in ((0, 512), (512, 1024), (1024, 1152)):
                            nc.tensor.matmul(
                                wp[:, s0:s1], lhs, w2_t[:, c0 + s0:c0 + s1],
                                start=True, stop=True)
                        nc.scalar.copy(
                            w_sb[:, b * WNUMEL + c0:b * WNUMEL + c0 + 1152],
                            wp[:])
                wv = w_sb[:].rearrange("p (b n) -> p b n", b=NB)    # [128,8,2304]

                # ---- per-edge contraction (batched muls + bf16 fold trees),
                # emitted per block-range so tile 0 can start in half-waves
                a0 = work.tile([BLK, NB * 48], bf16, tag="a0")
                a0v = a0[:].rearrange("p (b u) -> p b u", b=NB)
                tbv = work.tile([BLK, NB * 48], bf16, tag="tbv")
                tbvv = tbv[:].rearrange("p (b u i) -> p b u i", b=NB, u=16)
                x1sT = work.tile([BLK, NB * 48], bf16, tag="x1sT")
                x1sTv = x1sT[:].rearrange("p (b k u) -> p b k u", b=NB, k=3)
                outblk = outsp.tile([BLK, NB * 32], bf16, tag="outblk")
                obv = outblk[:].rearrange("p (b f) -> p b f", b=NB)
                tmp011 = tmpp.tile([BLK, NB * 512], bf16, tag="tmp011")
                t011 = tmp011[:].rearrange("p (b w u) -> p b w u", b=NB, w=16)
                g16 = tmpp.tile([BLK, NB * 256], bf16, tag="g16")
                u16 = g16[:].rearrange("p (b w u) -> p b w u", b=NB, w=16)
                g8 = tmpp.tile([BLK, NB * 128], bf16, tag="g8")
                u8 = g8[:].rearrange("p (b w u) -> p b w u", b=NB, w=16)
                g4 = tmpp.tile([BLK, NB * 64], bf16, tag="g4")
                u4 = g4[:].rearrange("p (b w u) -> p b w u", b=NB, w=16)
                g2 = tmpp.tile([BLK, NB * 32], bf16, tag="g2")
                u2 = g2[:].rearrange("p (b w u) -> p b w u", b=NB, w=16)
                cvec = work.tile([BLK, NB * 16], bf16, tag="cvec")
                cv = cvec[:].rearrange("p (b w) -> p b w", b=NB)
                tmp101 = tmpp.tile([BLK, NB * 3 * 256], bf16, tag="tmp101")
                t101 = tmp101[:].rearrange(
                    "p (b k w u) -> p b k w u", b=NB, k=3, w=16)
                h8 = tmpp.tile([BLK, NB * 3 * 128], bf16, tag="h8")
                q8 = h8[:].rearrange("p (b k w u) -> p b k w u", b=NB, k=3, w=16)
                h4 = tmpp.tile([BLK, NB * 3 * 64], bf16, tag="h4")
                q4 = h4[:].rearrange("p (b k w u) -> p b k w u", b=NB, k=3, w=16)
                h2 = tmpp.tile([BLK, NB * 3 * 32], bf16, tag="h2")
                q2 = h2[:].rearrange("p (b k w u) -> p b k w u", b=NB, k=3, w=16)
                dd = work.tile([BLK, NB * 48], bf16, tag="dd")
                ddv = dd[:].rearrange("p (b k w) -> p b k w", b=NB, k=3)
                tcx = work.tile([BLK, NB * 48], bf16, tag="tcx")
                tcv = tcx[:].rearrange("p (b k w) -> p b k w", b=NB, k=3)
                out1c = work.tile([BLK, NB * 48], bf16, tag="out1c")
                o1v = out1c[:].rearrange("p (b k w) -> p b k w", b=NB, k=3)
                tmp00 = tmpp.tile([BLK, NB * 1536], bf16, tag="tmp00")
                t00 = tmp00[:].rearrange("p (b w u) -> p b w u", b=NB, w=32)
                f24 = tmpp.tile([BLK, NB * 768], bf16, tag="f24")
                v24 = f24[:].rearrange("p (b w u) -> p b w u", b=NB, w=32)
                f12 = tmpp.tile([BLK, NB * 384], bf16, tag="f12")
                v12 = f12[:].rearrange("p (b w u) -> p b w u", b=NB, w=32)
                f6 = tmpp.tile([BLK, NB * 192], bf16, tag="f6")
                v6 = f6[:].rearrange("p (b w u) -> p b w u", b=NB, w=32)
                f3 = work.tile([BLK, NB * 96], bf16, tag="f3")
                v3 = f3[:].rearrange("p (b w u) -> p b w u", b=NB, w=32)
                f1 = work.tile([BLK, NB * 32], bf16, tag="f1")
                v1 = f1[:].rearrange("p (b w) -> p b w", b=NB)

                def emit_dve(bs, be):
                    nb = be - bs
                    s = slice(bs, be)
                    # chain-head preps on gpsimd (idle engine; 1x on DVE
                    # anyway due to innermost-0 broadcast APs)
                    nc.gpsimd.tensor_tensor(
                        a0v[:, s, 0:32], x1v[:, s, 0:32],
                        x2v[:, s, 0:1].broadcast_to((BLK, nb, 32)), MULT)
                    nc.gpsimd.tensor_tensor(
                        tbvv[:, s], x1v[:, s, 32:80].rearrange(
                            "p b (u i) -> p b u i", i=3),
                        x2v[:, s, 1:4].unsqueeze(2).broadcast_to(
                            (BLK, nb, 16, 3)), MULT)
                    nc.vector.tensor_reduce(
                        a0v[:, s, 32:48], tbvv[:, s], AXX, ADD)
                    nc.gpsimd.tensor_tensor(
                        x1sTv[:, s], x1v[:, s, 32:80].rearrange(
                            "p b (u k) -> p b k u", k=3),
                        x2v[:, s, 0:1].unsqueeze(3).broadcast_to(
                            (BLK, nb, 3, 16)), MULT)

                    # path011: c[b,w] = sum_u x1_0[b,u] * w[b, w*32+u]
                    nc.vector.tensor_tensor(
                        t011[:, s],
                        wv[:, s, 0:512].rearrange("p b (w u) -> p b w u", w=16),
                        x1v[:, s, 0:32].unsqueeze(2).broadcast_to(
                            (BLK, nb, 16, 32)), MULT)
                    nc.vector.tensor_tensor(u16[:, s], t011[:, s, :, 0:16],
                                            t011[:, s, :, 16:32], ADD)
                    nc.vector.tensor_tensor(u8[:, s], u16[:, s, :, 0:8],
                                            u16[:, s, :, 8:16], ADD)
                    nc.vector.tensor_tensor(u4[:, s], u8[:, s, :, 0:4],
                                            u8[:, s, :, 4:8], ADD)
                    nc.vector.tensor_tensor(u2[:, s], u4[:, s, :, 0:2],
                                            u4[:, s, :, 2:4], ADD)
                    nc.vector.tensor_tensor(cv[:, s], u2[:, s, :, 0],
                                            u2[:, s, :, 1], ADD)

                    # path101: d[b,k,w] = sum_u x1sT[b,k,u] * w[b, 512+w*16+u]
                    nc.vector.tensor_tensor(
                        t101[:, s],
                        wv[:, s, 512:768].rearrange("p b (w u) -> p b w u", w=16)
                            .unsqueeze(2).broadcast_to((BLK, nb, 3, 16, 16)),
                        x1sTv[:, s].unsqueeze(3).broadcast_to(
                            (BLK, nb, 3, 16, 16)), MULT)
                    nc.vector.tensor_tensor(q8[:, s], t101[:, s, :, :, 0:8],
                                            t101[:, s, :, :, 8:16], ADD)
                    nc.vector.tensor_tensor(q4[:, s], q8[:, s, :, :, 0:4],
                                            q8[:, s, :, :, 4:8], ADD)
                    nc.vector.tensor_tensor(q2[:, s], q4[:, s, :, :, 0:2],
                                            q4[:, s, :, :, 2:4], ADD)
                    nc.vector.tensor_tensor(ddv[:, s], q2[:, s, :, :, 0],
                                            q2[:, s, :, :, 1], ADD)

                    # out1[b,k,w] = x2_1[b,k]*c[b,w] + d[b,k,w] (contiguous
                    # [k,w]; the ACT outF copy does the (k,w)->(w,k) shuffle)
                    nc.vector.tensor_tensor(
                        tcv[:, s],
                        cv[:, s].unsqueeze(2).broadcast_to((BLK, nb, 3, 16)),
                        x2v[:, s, 1:4].unsqueeze(3).broadcast_to(
                            (BLK, nb, 3, 16)), MULT)
                    nc.vector.tensor_tensor(o1v[:, s], tcv[:, s], ddv[:, s],
                                            ADD)

                    # path00/110: out0[b,w] = sum_u a0[b,u] * w[b, 768+w*48+u]
                    nc.vector.tensor_tensor(
                        t00[:, s],
                        wv[:, s, 768:2304].rearrange("p b (w u) -> p b w u",
                                                     w=32),
                        a0v[:, s].unsqueeze(2).broadcast_to((BLK, nb, 32, 48)),
                        MULT)
                    nc.vector.tensor_tensor(v24[:, s], t00[:, s, :, 0:24],
                                            t00[:, s, :, 24:48], ADD)
                    nc.vector.tensor_tensor(v12[:, s], v24[:, s, :, 0:12],
                                            v24[:, s, :, 12:24], ADD)
                    nc.vector.tensor_tensor(v6[:, s], v12[:, s, :, 0:6],
                                            v12[:, s, :, 6:12], ADD)
                    nc.vector.tensor_tensor(v3[:, s], v6[:, s, :, 0:3],
                                            v6[:, s, :, 3:6], ADD)
                    nc.vector.tensor_tensor(v1[:, s], v3[:, s, :, 0],
                                            v3[:, s, :, 1], ADD)
                    nc.vector.tensor_tensor(obv[:, s, 0:32], v1[:, s],
                                            v3[:, s, :, 2], ADD)

                # finer waves early on so the DVE starts as soon as the first
                # blocks' evictions land (ramp), full-tile batching later
                if t == 0:
                    emit_dve(0, 2)
                    emit_dve(2, 4)
                    emit_dve(4, NB)
                elif t == 1:
                    emit_dve(0, NB // 2)
                    emit_dve(NB // 2, NB)
                else:
                    emit_dve(0, NB)

                outF = outsp.tile([BLK, NB * 80], f32, tag="outF")
                oFv = outF[:].rearrange("p (b f) -> p b f", b=NB)
                # split the last tile's store so the first half's copy + DMA
                # overlap the second half's remaining DVE work (shrinks tail)
                oranges = ((0, NB // 2), (NB // 2, NB)) if t == N_TILES - 1 \
                    else ((0, NB),)
                for bs, be in oranges:
                    nc.scalar.copy(oFv[:, bs:be, 0:32], obv[:, bs:be])
                    nc.scalar.copy(
                        oFv[:, bs:be, 32:80].rearrange(
                            "p b (w k) -> p b k w", k=3),
                        o1v[:, bs:be])
                    nc.sync.dma_start(
                        out_d[e0 + bs * BLK:e0 + be * BLK].rearrange(
                            "(b p) f -> p b f", p=BLK),
                        oFv[:, bs:be])

    nc.finalize()
    return nc


def kernel(fea_in1, fea_in2, fea_weight, W1, W2):
    import os
    import sys
    if os.environ.get("JAX_PLATFORMS") == "cpu" and "jax" not in sys.modules:
        # the NEFF executes through the axon/neuron jax backend
        del os.environ["JAX_PLATFORMS"]
    from concourse.bass_utils import run_bass_kernel_spmd

    if "nc" not in _NC_CACHE:
        _NC_CACHE["nc"] = _build()
    nc = _NC_CACHE["nc"]

    W1n = np.ascontiguousarray(np.asarray(W1, np.float32) / math.sqrt(FC_IN))
    W2n = _w2_prep(np.asarray(W2))
    fea_in1 = np.ascontiguousarray(np.asarray(fea_in1), dtype=np.float32)
    fea_in2 = np.ascontiguousarray(np.asarray(fea_in2), dtype=np.float32)
    fea_weight = np.ascontiguousarray(np.asarray(fea_weight), dtype=np.float32)

    in_maps = []
    for c in range(N_CORES):
        sl = slice(c * E_CORE, (c + 1) * E_CORE)
        in_maps.append({
            "fea_in1": fea_in1[sl],
            "fea_in2": fea_in2[sl],
            "fea_weight": fea_weight[sl],
            "W1n": W1n,
            "W2n": W2n,
        })
    res = run_bass_kernel_spmd(nc, in_maps, list(range(N_CORES)))
    return np.concatenate([res.results[c]["out"] for c in range(N_CORES)], axis=0)


# revision 40
# speedup vs baseline: 1.0006x; 1.0006x over previous
"""Trainium2 Bass kernel for nn_EquiConv (e3nn-style tensor product with
per-edge generated weights), data-parallel over edges on 8 NeuronCores.

Per 1024-edge tile (8 blocks of 128 edges on partitions):
  PE : fwT = transpose(fw) (bf16); hT = W1n^T @ fwT (bf16); per block:
       w = hT-slice^T @ W2n' (bf16, 1024-wide moving) -> PSUM f32
  ACT: h = silu(hT); evacuate w PSUM -> SBUF bf16 (2x 1152 per block)
  DVE: tile-batched broadcast-muls + bf16 fold-trees (2x mode), per-edge
       factors via step-0 broadcast APs; all path constants folded into W2n'.
  GPS: path101 product + first fold level (offload from DVE).

W2n' host prep: scaled by SILU_NORM/sqrt(64), per-path constants
(pw00, pw110/sqrt3, pw011/sqrt3, pw101/sqrt3) folded into the respective
column blocks, then permuted to w-major order per path, with path011 and
path101 FIRST so they only depend on the first eviction chunk (cols
0:1152) and their DVE chains can start before the tile's w00 evictions
finish:
  path011   : col w*32 + u          (cols    0: 512)
  path101   : 512 + w*16 + u        (cols  512: 768)
  path00/110: 768 + w*48 + u        (cols  768:2304; u<32 -> w00 else w110)
"""
import math

import numpy as np

E_TOTAL = 65536
N_CORES = 8
E_CORE = E_TOTAL // N_CORES        # 8192
TILE_E = 1024
BLK = 128
NB = TILE_E // BLK                 # 8 blocks per tile
N_TILES = E_CORE // TILE_E         # 8
MUL0, MUL1 = 32, 16
FC_IN, FC_HID = 64, 64
WNUMEL = 2304
SILU_NORM = 1.6790
ISQRT3 = 1.0 / math.sqrt(3.0)
PW00 = math.sqrt(1.0 / (MUL0 * 2))
PW110I3 = math.sqrt(1.0 / (MUL1 * 2)) * ISQRT3
PW011I3 = math.sqrt(3.0 / (MUL0 * 2)) * ISQRT3
PW101I3 = math.sqrt(3.0 / (MUL1 * 2)) * ISQRT3

GPS_T101 = False                   # gpsimd TT measured ~6x slower than DVE; keep off

_NC_CACHE = {}


def _w2_prep(W2):
    """Scale + fold path constants + permute to w-major. Returns [64, 2304]."""
    W2n = W2.astype(np.float64) * (SILU_NORM / math.sqrt(FC_HID))
    W2n[:, 0:1024] *= PW00
    W2n[:, 1024:1536] *= PW110I3
    W2n[:, 1536:2048] *= PW011I3
    W2n[:, 2048:2304] *= PW101I3
    old = np.empty(WNUMEL, np.int64)
    for w in range(16):
        for u in range(32):
            old[w * 32 + u] = 1536 + u * 16 + w
    for w in range(16):
        for u in range(16):
            old[512 + w * 16 + u] = 2048 + u * 16 + w
    for w in range(32):
        for u in range(48):
            old[768 + w * 48 + u] = (u * 32 + w) if u < 32 \
                else (1024 + (u - 32) * 32 + w)
    return np.ascontiguousarray(W2n[:, old]).astype(np.float32)


def _build():
    import concourse.tile as tile
    from concourse import bacc, mybir
    from concourse.masks import make_identity

    f32 = mybir.dt.float32
    bf16 = mybir.dt.bfloat16
    MULT = mybir.AluOpType.mult
    ADD = mybir.AluOpType.add
    AXX = mybir.AxisListType.X

    nc = bacc.Bacc("TRN2", target_bir_lowering=False, debug=False)
    fea_in1 = nc.declare_dram_parameter("fea_in1", [E_CORE, 80], f32, isOutput=False)
    fea_in2 = nc.declare_dram_parameter("fea_in2", [E_CORE, 4], f32, isOutput=False)
    fea_w = nc.declare_dram_parameter("fea_weight", [E_CORE, 64], f32, isOutput=False)
    W1n = nc.declare_dram_parameter("W1n", [64, 64], f32, isOutput=False)
    W2n = nc.declare_dram_parameter("W2n", [64, WNUMEL], f32, isOutput=False)
    out_d = nc.declare_dram_parameter("out", [E_CORE, 80], f32, isOutput=True)

    with tile.TileContext(nc) as tc, nc.allow_low_precision("bf16 contraction"):
        with (
            tc.tile_pool(name="consts", bufs=1) as consts,
            tc.tile_pool(name="ins", bufs=3) as insp,
            tc.tile_pool(name="mid", bufs=2) as mid,
            tc.tile_pool(name="wsb", bufs=2) as wsbp,
            tc.tile_pool(name="work", bufs=3) as work,
            tc.tile_pool(name="tmp", bufs=1) as tmpp,
            tc.tile_pool(name="outs", bufs=2) as outsp,
            tc.tile_pool(name="ps_w", bufs=2, space="PSUM") as ps_w,
            tc.tile_pool(name="ps_s", bufs=1, space="PSUM") as ps_s,
        ):
            # ident first: it gates the PE warm-up, so it must not queue
            # behind the large W2n cast-DMA on the gpsimd queue
            ident = consts.tile([128, 128], f32)
            make_identity(nc, ident)
            w1_t = consts.tile([64, 64], bf16)
            nc.gpsimd.dma_start(w1_t[:], W1n[:])
            # W2n load split so mm2's first chunk (cols 0:1152) is usable
            # before the whole matrix lands
            w2_t = consts.tile([64, WNUMEL], bf16)
            nc.gpsimd.dma_start(w2_t[:, 0:1152], W2n[:, 0:1152])
            nc.gpsimd.dma_start(w2_t[:, 1152:WNUMEL], W2n[:, 1152:WNUMEL])

            # PE warm-up: start PE activity early to un-throttle the HAM
            # clock gate (1.2 -> 2.4 GHz); tile 0's real transposes continue
            # the activity stream, so a short burst suffices.
            warm = ps_s.tile([64, 512], f32, tag="fwT")
            for i in range(10):
                nc.tensor.transpose(
                    warm[:, (i % 4) * 128:(i % 4 + 1) * 128],
                    ident[:, 0:64], ident[:])

            for t in range(N_TILES):
                e0 = t * TILE_E
                # ---- batched input loads (bf16 via DMA convert) ----
                fwB = insp.tile([BLK, NB * 64], f32, tag="fwB")
                nc.sync.dma_start(
                    fwB[:].rearrange("p (b f) -> p b f", b=NB),
                    fea_w[e0:e0 + TILE_E].rearrange("(b p) f -> p b f", p=BLK))
                x1B = insp.tile([BLK, NB * 80], bf16, tag="x1B")
                nc.gpsimd.dma_start(
                    x1B[:].rearrange("p (b f) -> p b f", b=NB),
                    fea_in1[e0:e0 + TILE_E].rearrange("(b p) f -> p b f", p=BLK))
                x2B = insp.tile([BLK, NB * 4], f32, tag="x2B")
                nc.sync.dma_start(
                    x2B[:].rearrange("p (b f) -> p b f", b=NB),
                    fea_in2[e0:e0 + TILE_E].rearrange("(b p) f -> p b f", p=BLK))

                # views over the batched per-edge features
                x1v = x1B[:].rearrange("p (b f) -> p b f", b=NB)    # [128,8,80]
                x2v = x2B[:].rearrange("p (b f) -> p b f", b=NB)    # [128,8,4]

                # ---- fw transpose + mm1 + silu (per 512-half) ----
                fwT_sb = mid.tile([64, TILE_E], bf16, tag="fwT_sb")
                h_sb = mid.tile([64, TILE_E], bf16, tag="h_sb")
                for hf in range(TILE_E // 512):
                    fwT_ps = ps_s.tile([64, 512], f32, tag="fwT")
                    for b in range(4):
                        nc.tensor.transpose(
                            fwT_ps[:, b * BLK:(b + 1) * BLK],
                            fwB[:, (hf * 4 + b) * 64:(hf * 4 + b + 1) * 64],
                            ident[:])
                    nc.scalar.copy(
                        fwT_sb[:, hf * 512:(hf + 1) * 512], fwT_ps[:])
                    h_ps = ps_s.tile([64, 512], f32, tag="h")
                    nc.tensor.matmul(
                        h_ps[:], w1_t[:], fwT_sb[:, hf * 512:(hf + 1) * 512],
                        start=True, stop=True)
                    nc.scalar.activation(
                        h_sb[:, hf * 512:(hf + 1) * 512], h_ps[:],
                        mybir.ActivationFunctionType.Silu)

                # ---- mm2 (bf16) + eviction (2x 1152 per block) ----
                # all wpA chunks (cols 0:1152) first: paths 011+101 live
                # entirely in wpA, so their DVE chains unlock after 8
                # evictions instead of 15
                w_sb = wsbp.tile([BLK, NB * WNUMEL], bf16, tag="w_sb")
                for half in range(2):
                    c0 = half * 1152
                    for b in range(NB):
                        lhs = h_sb[:, b * BLK:(b + 1) * BLK]
                        wp = ps_w.tile([BLK, 1152], f32, tag="wp")
                        for s0, s1 in ((0, 512), (512, 1024), (1024, 1152)):
                            nc.tensor.matmul(
                                wp[:, s0:s1], lhs, w2_t[:, c0 + s0:c0 + s1],
                                start=True, stop=True)
                        nc.scalar.copy(
                            w_sb[:, b * WNUMEL + c0:b * WNUMEL + c0 + 1152],
                            wp[:])
                wv = w_sb[:].rearrange("p (b n) -> p b n", b=NB)    # [128,8,2304]

                # ---- per-edge contraction (batched muls + bf16 fold trees),
                # emitted per block-range so tile 0 can start in half-waves
                a0 = work.tile([BLK, NB * 48], bf16, tag="a0")
                a0v = a0[:].rearrange("p (b u) -> p b u", b=NB)
                tbv = work.tile([BLK, NB * 48], bf16, tag="tbv")
                tbvv = tbv[:].rearrange("p (b u i) -> p b u i", b=NB, u=16)
                x1sT = work.tile([BLK, NB * 48], bf16, tag="x1sT")
                x1sTv = x1sT[:].rearrange("p (b k u) -> p b k u", b=NB, k=3)
                outblk = outsp.tile([BLK, NB * 32], bf16, tag="outblk")
                obv = outblk[:].rearrange("p (b f) -> p b f", b=NB)
                tmp011 = tmpp.tile([BLK, NB * 512], bf16, tag="tmp011")
                t011 = tmp011[:].rearrange("p (b w u) -> p b w u", b=NB, w=16)
                g16 = tmpp.tile([BLK, NB * 256], bf16, tag="g16")
                u16 = g16[:].rearrange("p (b w u) -> p b w u", b=NB, w=16)
                g8 = tmpp.tile([BLK, NB * 128], bf16, tag="g8")
                u8 = g8[:].rearrange("p (b w u) -> p b w u", b=NB, w=16)
                g4 = tmpp.tile([BLK, NB * 64], bf16, tag="g4")
                u4 = g4[:].rearrange("p (b w u) -> p b w u", b=NB, w=16)
                g2 = tmpp.tile([BLK, NB * 32], bf16, tag="g2")
                u2 = g2[:].rearrange("p (b w u) -> p b w u", b=NB, w=16)
                cvec = work.tile([BLK, NB * 16], bf16, tag="cvec")
                cv = cvec[:].rearrange("p (b w) -> p b w", b=NB)
                tmp101 = tmpp.tile([BLK, NB * 3 * 256], bf16, tag="tmp101")
                t101 = tmp101[:].rearrange(
                    "p (b k w u) -> p b k w u", b=NB, k=3, w=16)
                h8 = tmpp.tile([BLK, NB * 3 * 128], bf16, tag="h8")
                q8 = h8[:].rearrange("p (b k w u) -> p b k w u", b=NB, k=3, w=16)
                h4 = tmpp.tile([BLK, NB * 3 * 64], bf16, tag="h4")
                q4 = h4[:].rearrange("p (b k w u) -> p b k w u", b=NB, k=3, w=16)
                h2 = tmpp.tile([BLK, NB * 3 * 32], bf16, tag="h2")
                q2 = h2[:].rearrange("p (b k w u) -> p b k w u", b=NB, k=3, w=16)
                dd = work.tile([BLK, NB * 48], bf16, tag="dd")
                ddv = dd[:].rearrange("p (b k w) -> p b k w", b=NB, k=3)
                tcx = work.tile([BLK, NB * 48], bf16, tag="tcx")
                tcv = tcx[:].rearrange("p (b k w) -> p b k w", b=NB, k=3)
                out1c = work.tile([BLK, NB * 48], bf16, tag="out1c")
                o1v = out1c[:].rearrange("p (b k w) -> p b k w", b=NB, k=3)
                tmp00 = tmpp.tile([BLK, NB * 1536], bf16, tag="tmp00")
                t00 = tmp00[:].rearrange("p (b w u) -> p b w u", b=NB, w=32)
                f24 = tmpp.tile([BLK, NB * 768], bf16, tag="f24")
                v24 = f24[:].rearrange("p (b w u) -> p b w u", b=NB, w=32)
                f12 = tmpp.tile([BLK, NB * 384], bf16, tag="f12")
                v12 = f12[:].rearrange("p (b w u) -> p b w u", b=NB, w=32)
                f6 = tmpp.tile([BLK, NB * 192], bf16, tag="f6")
                v6 = f6[:].rearrange("p (b w u) -> p b w u", b=NB, w=32)
                f3 = work.tile([BLK, NB * 96], bf16, tag="f3")
                v3 = f3[:].rearrange("p (b w u) -> p b w u", b=NB, w=32)
                f1 = work.tile([BLK, NB * 32], bf16, tag="f1")
                v1 = f1[:].rearrange("p (b w) -> p b w", b=NB)

                def emit_dve(bs, be):
                    nb = be - bs
                    s = slice(bs, be)
                    # chain-head preps on gpsimd (idle engine; 1x on DVE
                    # anyway due to innermost-0 broadcast APs)
                    nc.gpsimd.tensor_tensor(
                        a0v[:, s, 0:32], x1v[:, s, 0:32],
                        x2v[:, s, 0:1].broadcast_to((BLK, nb, 32)), MULT)
                    nc.gpsimd.tensor_tensor(
                        tbvv[:, s], x1v[:, s, 32:80].rearrange(
                            "p b (u i) -> p b u i", i=3),
                        x2v[:, s, 1:4].unsqueeze(2).broadcast_to(
                            (BLK, nb, 16, 3)), MULT)
                    nc.vector.tensor_reduce(
                        a0v[:, s, 32:48], tbvv[:, s], AXX, ADD)
                    nc.gpsimd.tensor_tensor(
                        x1sTv[:, s], x1v[:, s, 32:80].rearrange(
                            "p b (u k) -> p b k u", k=3),
                        x2v[:, s, 0:1].unsqueeze(3).broadcast_to(
                            (BLK, nb, 3, 16)), MULT)

                    # path011: c[b,w] = sum_u x1_0[b,u] * w[b, w*32+u]
                    nc.vector.tensor_tensor(
                        t011[:, s],
                        wv[:, s, 0:512].rearrange("p b (w u) -> p b w u", w=16),
                        x1v[:, s, 0:32].unsqueeze(2).broadcast_to(
                            (BLK, nb, 16, 32)), MULT)
                    nc.vector.tensor_tensor(u16[:, s], t011[:, s, :, 0:16],
                                            t011[:, s, :, 16:32], ADD)
                    nc.vector.tensor_tensor(u8[:, s], u16[:, s, :, 0:8],
                                            u16[:, s, :, 8:16], ADD)
                    nc.vector.tensor_tensor(u4[:, s], u8[:, s, :, 0:4],
                                            u8[:, s, :, 4:8], ADD)
                    nc.vector.tensor_tensor(u2[:, s], u4[:, s, :, 0:2],
                                            u4[:, s, :, 2:4], ADD)
                    nc.vector.tensor_tensor(cv[:, s], u2[:, s, :, 0],
                                            u2[:, s, :, 1], ADD)

                    # path101: d[b,k,w] = sum_u x1sT[b,k,u] * w[b, 512+w*16+u]
                    nc.vector.tensor_tensor(
                        t101[:, s],
                        wv[:, s, 512:768].rearrange("p b (w u) -> p b w u", w=16)
                            .unsqueeze(2).broadcast_to((BLK, nb, 3, 16, 16)),
                        x1sTv[:, s].unsqueeze(3).broadcast_to(
                            (BLK, nb, 3, 16, 16)), MULT)
                    nc.vector.tensor_tensor(q8[:, s], t101[:, s, :, :, 0:8],
                                            t101[:, s, :, :, 8:16], ADD)
                    nc.vector.tensor_tensor(q4[:, s], q8[:, s, :, :, 0:4],
                                            q8[:, s, :, :, 4:8], ADD)
                    nc.vector.tensor_tensor(q2[:, s], q4[:, s, :, :, 0:2],
                                            q4[:, s, :, :, 2:4], ADD)
                    nc.vector.tensor_tensor(ddv[:, s], q2[:, s, :, :, 0],
                                            q2[:, s, :, :, 1], ADD)

                    # out1[b,k,w] = x2_1[b,k]*c[b,w] + d[b,k,w] (contiguous
                    # [k,w]; the ACT outF copy does the (k,w)->(w,k) shuffle)
                    nc.vector.tensor_tensor(
                        tcv[:, s],
                        cv[:, s].unsqueeze(2).broadcast_to((BLK, nb, 3, 16)),
                        x2v[:, s, 1:4].unsqueeze(3).broadcast_to(
                            (BLK, nb, 3, 16)), MULT)
                    nc.vector.tensor_tensor(o1v[:, s], tcv[:, s], ddv[:, s],
                                            ADD)

                    # path00/110: out0[b,w] = sum_u a0[b,u] * w[b, 768+w*48+u]
                    nc.vector.tensor_tensor(
                        t00[:, s],
                        wv[:, s, 768:2304].rearrange("p b (w u) -> p b w u",
                                                     w=32),
                        a0v[:, s].unsqueeze(2).broadcast_to((BLK, nb, 32, 48)),
                        MULT)
                    nc.vector.tensor_tensor(v24[:, s], t00[:, s, :, 0:24],
                                            t00[:, s, :, 24:48], ADD)
                    nc.vector.tensor_tensor(v12[:, s], v24[:, s, :, 0:12],
                                            v24[:, s, :, 12:24], ADD)
                    nc.vector.tensor_tensor(v6[:, s], v12[:, s, :, 0:6],
                                            v12[:, s, :, 6:12], ADD)
                    nc.vector.tensor_tensor(v3[:, s], v6[:, s, :, 0:3],
                                            v6[:, s, :, 3:6], ADD)
                    nc.vector.tensor_tensor(v1[:, s], v3[:, s, :, 0],
                                            v3[:, s, :, 1], ADD)
                    nc.vector.tensor_tensor(obv[:, s, 0:32], v1[:, s],
                                            v3[:, s, :, 2], ADD)

                # finer waves early on so the DVE starts as soon as the first
                # blocks' evictions land (ramp), full-tile batching later
                if t == 0:
                    emit_dve(0, 2)
                    emit_dve(2, 4)
                    emit_dve(4, NB)
                elif t == 1:
                    emit_dve(0, NB // 2)
                    emit_dve(NB // 2, NB)
                else:
                    emit_dve(0, NB)

                outF = outsp.tile([BLK, NB * 80], f32, tag="outF")
                oFv = outF[:].rearrange("p (b f) -> p b f", b=NB)
                # split the last tile's store so the first half's copy + DMA
                # overlap the second half's remaining DVE work (shrinks tail)
                oranges = ((0, NB // 2), (NB // 2, NB)) if t == N_TILES - 1 \
                    else ((0, NB),)
                for bs, be in oranges:
                    nc.scalar.copy(oFv[:, bs:be, 0:32], obv[:, bs:be])
                    nc.scalar.copy(
                        oFv[:, bs:be, 32:80].rearrange(
                            "p b (w k) -> p b k w", k=3),
                        o1v[:, bs:be])
                    nc.sync.dma_start(
                        out_d[e0 + bs * BLK:e0 + be * BLK].rearrange(
                            "(b p) f -> p b f", p=BLK),
                        oFv[:, bs:be])

    nc.finalize()
    return nc


def kernel(fea_in1, fea_in2, fea_weight, W1, W2):
    import os
    import sys
    if os.environ.get("JAX_PLATFORMS") == "cpu" and "jax" not in sys.modules:
        # the NEFF executes through the axon/neuron jax backend
        del os.environ["JAX_PLATFORMS"]
    from concourse.bass_utils import run_bass_kernel_spmd

    if "nc" not in _NC_CACHE:
        _NC_CACHE["nc"] = _build()
    nc = _NC_CACHE["nc"]

    W1n = np.ascontiguousarray(np.asarray(W1, np.float32) / math.sqrt(FC_IN))
    W2n = _w2_prep(np.asarray(W2))
    fea_in1 = np.ascontiguousarray(np.asarray(fea_in1), dtype=np.float32)
    fea_in2 = np.ascontiguousarray(np.asarray(fea_in2), dtype=np.float32)
    fea_weight = np.ascontiguousarray(np.asarray(fea_weight), dtype=np.float32)

    in_maps = []
    for c in range(N_CORES):
        sl = slice(c * E_CORE, (c + 1) * E_CORE)
        in_maps.append({
            "fea_in1": fea_in1[sl],
            "fea_in2": fea_in2[sl],
            "fea_weight": fea_weight[sl],
            "W1n": W1n,
            "W2n": W2n,
        })
    res = run_bass_kernel_spmd(nc, in_maps, list(range(N_CORES)))
    return np.concatenate([res.results[c]["out"] for c in range(N_CORES)], axis=0)
